# revision 1
# baseline (speedup 1.0000x reference)
"""BiLSTM (B=64, L=256, D=512, H=512) on 8 Trainium2 NeuronCores.

Strategy: 8 cores = 2 directions x 4 batch-slices of 16 (weights replicated
per direction, sequential time loop local to each core — no cross-core
communication).  Backward-direction cores receive time-reversed x, so every
core runs the identical SPMD program.

Per-core program:
  Phase 1: xpart[token, 4H] = x_t @ Wx.T + bias as one large GEMM
           (full 128x128 PE utilization), staged to DRAM.
  Phase 2: 256 recurrence steps.  Per step:
    - gates psum tile (128, 512) holds 4 gate strips f/i/o/g at partition
      offsets {0,32,64,96} via column-tiled matmuls (tile_position), so the
      four (K=128, M=16, N=512) matmul groups stream concurrently through
      the four 32-column groups of the PE array;
    - xpart is injected with identity-stationary matmuls (start=True);
    - sigmoid/tanh on ScalarE (partition-shifted outputs), cell/hidden
      update on VectorE;
    - h (16, 512) is transposed back to 4 hT chunks (128, 16) with
      PE-transpose for use as the next step's stationary operand.
"""

import numpy as np

from concourse import tile, mybir, bacc
from concourse.bass_utils import run_bass_kernel_spmd
from concourse.masks import make_identity

FP = mybir.dt.float32
AF = mybir.ActivationFunctionType

B = 16        # local batch per core
L = 256       # timesteps
D = 512       # input dim
H = 512       # hidden
NG = 4 * H    # gate width
TOK = L * B   # tokens per core
NM = TOK // 128

_CACHED_NC = None


def _build():
    nc = bacc.Bacc("TRN2", target_bir_lowering=False, debug=False)

    xT = nc.dram_tensor("xT", [D, TOK], FP, kind="ExternalInput").ap()
    W = nc.dram_tensor("W", [D + H, NG], FP, kind="ExternalInput").ap()
    bias = nc.dram_tensor("bias", [1, NG], FP, kind="ExternalInput").ap()
    out_h = nc.dram_tensor("out_h", [L, B, H], FP, kind="ExternalOutput").ap()
    xp_dram = nc.dram_tensor("xp_dram", [NM, 128, NG], FP).ap()

    with tile.TileContext(nc, trace_sim=False) as tc:
        with tc.tile_pool(name="wpool", bufs=1) as wpool, \
             tc.tile_pool(name="cpool", bufs=1) as cpool:
            W_t = []
            for k in range(8):
                wt = wpool.tile([128, NG], FP, tag=f"w{k}", name=f"w{k}")
                nc.sync.dma_start(wt[:], W[128 * k:128 * (k + 1), :])
                W_t.append(wt)
            bias_t = wpool.tile([1, NG], FP)
            nc.sync.dma_start(bias_t[:], bias[:, :])
            ones_t = cpool.tile([1, 128], FP)
            nc.vector.memset(ones_t[:, :], 1.0)
            ident = cpool.tile([B, B], FP)
            make_identity(nc, ident[:, :])

            # Phase 1: xpart GEMM
            with tc.tile_pool(name="p1x", bufs=3) as p1x, \
                 tc.tile_pool(name="p1ps", bufs=4, space="PSUM") as p1ps:
                for m in range(NM):
                    xm = p1x.tile([128, 4, 128], FP, tag="xm", name="xm")
                    for k in range(4):
                        nc.sync.dma_start(
                            xm[:, k, :],
                            xT[128 * k:128 * (k + 1), 128 * m:128 * (m + 1)])
                    for n in range(4):
                        ps = p1ps.tile([128, 512], FP, tag="ps1", name="ps1")
                        for k in range(4):
                            nc.tensor.matmul(
                                ps[:, :], xm[:, k, :],
                                W_t[k][:, 512 * n:512 * (n + 1)],
                                start=(k == 0), stop=False)
                        nc.tensor.matmul(
                            ps[:, :], ones_t[:, :],
                            bias_t[:, 512 * n:512 * (n + 1)],
                            start=False, stop=True)
                        sb = p1x.tile([128, 512], FP, tag="sb1", name="sb1")
                        nc.scalar.copy(sb[:, :], ps[:, :])
                        nc.sync.dma_start(
                            xp_dram[m, :, 512 * n:512 * (n + 1)], sb[:, :])

            # Phase 2: recurrence
            with tc.tile_pool(name="xpp", bufs=4) as xpp, \
                 tc.tile_pool(name="st", bufs=2) as st, \
                 tc.tile_pool(name="ch", bufs=2) as ch, \
                 tc.tile_pool(name="gps", bufs=2, space="PSUM") as gps, \
                 tc.tile_pool(name="tps", bufs=2, space="PSUM") as tps:

                c_prev = st.tile([B, H], FP, tag="c", name="c0")
                nc.vector.memset(c_prev[:, :], 0.0)
                hT_prev = []
                for kc in range(4):
                    t0 = st.tile([128, B], FP, tag=f"hT{kc}", name=f"hT{kc}_0")
                    nc.vector.memset(t0[:, :], 0.0)
                    hT_prev.append(t0)

                for t in range(L):
                    xp_t = xpp.tile([B, NG], FP, tag="xp", name="xp")
                    nc.sync.dma_start(
                        xp_t[:],
                        xp_dram[t // 8, B * (t % 8):B * (t % 8) + B, :])

                    P = gps.tile([128, 512], FP, tag="P", name="P")
                    for j in range(4):
                        nc.tensor.matmul(
                            P[32 * j:32 * j + B, :], ident[:, :],
                            xp_t[:, 512 * j:512 * (j + 1)],
                            start=True, stop=False, tile_position=(0, 32 * j))
                    for k in range(4):
                        for j in range(4):
                            nc.tensor.matmul(
                                P[32 * j:32 * j + B, :], hT_prev[k][:, :],
                                W_t[4 + k][:, 512 * j:512 * (j + 1)],
                                start=False, stop=(k == 3),
                                tile_position=(0, 32 * j))

                    # strips: f@0:16, i@32:48, o@64:80, g@96:112
                    s = ch.tile([80, H], FP, tag="s", name="s")
                    nc.scalar.activation(s[:, :], P[0:80, :], AF.Sigmoid)
                    g_t = ch.tile([48, H], FP, tag="g_t", name="g_t")
                    nc.scalar.activation(g_t[32:48, :], P[96:112, :], AF.Tanh)
                    t1 = ch.tile([B, H], FP, tag="t1", name="t1")
                    nc.vector.tensor_mul(t1[:, :], s[0:B, :], c_prev[:, :])
                    t2 = ch.tile([B, H], FP, tag="t2", name="t2")
                    nc.vector.tensor_mul(t2[:, :], s[32:48, :], g_t[32:48, :])
                    c_new = st.tile([B, H], FP, tag="c", name="c")
                    nc.vector.tensor_add(c_new[:, :], t1[:, :], t2[:, :])
                    th = ch.tile([80, H], FP, tag="th", name="th")
                    nc.scalar.activation(th[64:80, :], c_new[:, :], AF.Tanh)
                    h_new = st.tile([B, H], FP, tag="h", name="h")
                    nc.vector.tensor_mul(h_new[:, :], s[64:80, :], th[64:80, :])

                    nc.sync.dma_start(out_h[t, :, :], h_new[:, :])

                    hT_new = []
                    pst = tps.tile([128, 64], FP, tag="pst", name="pst")
                    for kc in range(4):
                        nc.tensor.transpose(
                            pst[:, 16 * kc:16 * (kc + 1)],
                            h_new[:, 128 * kc:128 * (kc + 1)], ident[:, :])
                        hTn = st.tile([128, B], FP, tag=f"hT{kc}", name=f"hT{kc}")
                        nc.vector.tensor_copy(
                            hTn[:, :], pst[:, 16 * kc:16 * (kc + 1)])
                        hT_new.append(hTn)
                    c_prev = c_new
                    hT_prev = hT_new
    nc.compile()
    return nc


def _host_prepare(x_full, weights, direction, bslice):
    xs = x_full[bslice]
    if direction == "bw":
        xs = xs[:, ::-1, :]
    xT = np.ascontiguousarray(xs.transpose(2, 1, 0).reshape(D, TOK))
    Wc = np.concatenate(
        [weights[f"W_{direction}_{n}"].T for n in "fiog"], axis=1)
    bc = np.concatenate(
        [weights[f"b_{direction}_{n}"] for n in "fiog"])[None, :]
    return {"xT": xT.astype(np.float32),
            "W": np.ascontiguousarray(Wc).astype(np.float32),
            "bias": np.ascontiguousarray(bc).astype(np.float32)}


def kernel(**inputs):
    global _CACHED_NC
    inputs = {k: np.asarray(v) for k, v in inputs.items()}
    x = inputs["x"]
    Bx, Lx, _ = x.shape
    assert (Bx, Lx) == (64, L)

    if _CACHED_NC is None:
        _CACHED_NC = _build()
    nc = _CACHED_NC

    in_maps = []
    meta = []
    for ci in range(8):
        d = "fw" if ci < 4 else "bw"
        bs = (ci % 4) * B
        in_maps.append(_host_prepare(x, inputs, d, slice(bs, bs + B)))
        meta.append((d, bs))

    res = run_bass_kernel_spmd(nc, in_maps, core_ids=list(range(8)))

    hf = np.zeros((L, Bx, H), np.float32)
    hb = np.zeros((L, Bx, H), np.float32)
    for ci in range(8):
        d, bs = meta[ci]
        oh = res.results[ci]["out_h"]  # (L, 16, H), time-major
        if d == "fw":
            hf[:, bs:bs + B, :] = oh
        else:
            hb[:, bs:bs + B, :] = oh[::-1]

    # faithful to the reference: stack time-major, flatten, hstack, reshape
    flat = np.concatenate([hf.reshape(-1, H), hb.reshape(-1, H)], axis=1)
    return flat.reshape(Bx, Lx, 2 * H).astype(np.float32)



# revision 21
# speedup vs baseline: 1.3512x; 1.3512x over previous
"""BiLSTM (B=64, L=256, D=512, H=512) on 8 Trainium2 NeuronCores.

8 cores = 2 directions x 4 batch-slices of 16; weights replicated per
direction; backward cores get time-reversed x so one SPMD program serves
all cores.

v1 vs baseline:
  - all matmul operands bf16 (fp32 matmuls are 2-pass on TRN2; bf16 is
    single-pass) with fp32 PSUM accumulation; cell state c kept fp32.
  - xpart (x @ Wx.T + b for all tokens) kept resident in SBUF (16 MB bf16)
    instead of a 64 MB DRAM round-trip.
  - gate PSUM layout [128, 256]: partition = 32*gate + 16*h_half + batch,
    free = 256 H-half columns -> activations and vector ops see 256-free
    tiles instead of 512-free (ScalarE/DVE cost scales with free dim).
  - h produced in fp32 for output DMA; bf16 copy feeds 4 PE transposes ->
    one strided DVE copy into the zero-padded stationary store hT
    [128, 4, 32] (cols 0:16 zero so the h_half=1 matmul can use a 32-wide
    stationary whose first 16 columns contribute nothing).
"""

import numpy as np
import ml_dtypes

from concourse import tile, mybir, bacc
from concourse.bass_utils import run_bass_kernel_spmd
from concourse.masks import make_identity

FP = mybir.dt.float32
BF = mybir.dt.bfloat16
AF = mybir.ActivationFunctionType

B = 16        # local batch per core
L = 256       # timesteps
D = 512       # input dim
H = 512       # hidden
NG = 4 * H    # gate width
HH = H // 2   # h-half
TOK = L * B   # tokens per core
NM = TOK // 128

_CACHED_NC = None


def _build():
    nc = bacc.Bacc("TRN2", target_bir_lowering=False, debug=False)

    xT = nc.dram_tensor("xT", [D, TOK], BF, kind="ExternalInput").ap()
    W = nc.dram_tensor("W", [D + H, NG], BF, kind="ExternalInput").ap()
    bias = nc.dram_tensor("bias", [1, NG], BF, kind="ExternalInput").ap()
    # row-selector stationaries: cols 0:16 = S0, 16:32 = S16,
    # 32:64 = [0|S0], 64:96 = [0|S16]  (S_o selects moving rows o..o+16)
    idsel_d = nc.dram_tensor("idsel", [128, 96], BF, kind="ExternalInput").ap()
    id32_d = nc.dram_tensor("id32", [32, 32], FP, kind="ExternalInput").ap()
    out_h = nc.dram_tensor("out_h", [L, 2 * B, HH], FP, kind="ExternalOutput").ap()

    with tile.TileContext(nc, trace_sim=False) as tc:
        with tc.tile_pool(name="wpool", bufs=1) as wpool, \
             tc.tile_pool(name="xppool", bufs=1) as xppool, \
             tc.tile_pool(name="cpool", bufs=1) as cpool:
            W_t = []
            for k in range(8):
                wt = wpool.tile([128, NG], BF, tag=f"w{k}", name=f"w{k}")
                nc.sync.dma_start(wt[:], W[128 * k:128 * (k + 1), :])
                W_t.append(wt)
            bias_t = wpool.tile([1, NG], BF)
            nc.sync.dma_start(bias_t[:], bias[:, :])
            ones_t = cpool.tile([1, 128], BF)
            nc.vector.memset(ones_t[:, :], 1.0)
            idsel = cpool.tile([128, 96], BF, tag="idsel", name="idsel")
            nc.sync.dma_start(idsel[:, :], idsel_d[:, :])
            ident32 = cpool.tile([32, 32], FP, tag="ident32", name="ident32")
            nc.sync.dma_start(ident32[:, :], id32_d[:, :])

            # persistent SBUF xpart tiles (32 x [128, 2048] bf16)
            xp_t = [xppool.tile([128, NG], BF, tag=f"xp{m}", name=f"xp{m}")
                    for m in range(NM)]

            # state tiles (ping-pong)
            c_t = [cpool.tile([2 * B, HH], FP, tag=f"c{i}", name=f"c{i}")
                   for i in range(2)]
            hT_t = [cpool.tile([128, 4, 32], BF, tag=f"hT{i}", name=f"hT{i}")
                    for i in range(2)]
            nc.vector.memset(c_t[0][:, :], 0.0)
            nc.vector.memset(hT_t[0][:, :, :], 0.0)
            nc.vector.memset(hT_t[1][:, :, :], 0.0)

            # Phase 1: xpart GEMM (bf16), results stay in SBUF
            with tc.tile_pool(name="p1x", bufs=3) as p1x, \
                 tc.tile_pool(name="p1ps", bufs=4, space="PSUM") as p1ps:
                for m in range(NM):
                    xm = p1x.tile([128, 4, 128], BF, tag="xm", name="xm")
                    for k in range(4):
                        nc.sync.dma_start(
                            xm[:, k, :],
                            xT[128 * k:128 * (k + 1), 128 * m:128 * (m + 1)])
                    for n in range(4):
                        ps = p1ps.tile([128, 512], FP, tag="ps1", name="ps1")
                        for k in range(4):
                            nc.tensor.matmul(
                                ps[:, :], xm[:, k, :],
                                W_t[k][:, 512 * n:512 * (n + 1)],
                                start=(k == 0), stop=False)
                        nc.tensor.matmul(
                            ps[:, :], ones_t[:, :],
                            bias_t[:, 512 * n:512 * (n + 1)],
                            start=False, stop=True)
                        nc.vector.tensor_copy(
                            xp_t[m][:, 512 * n:512 * (n + 1)], ps[:, :])

            # Phase 2: recurrence
            with tc.tile_pool(name="ch", bufs=3) as ch, \
                 tc.tile_pool(name="hb", bufs=2) as hb, \
                 tc.tile_pool(name="gps", bufs=2, space="PSUM") as gps, \
                 tc.tile_pool(name="tps", bufs=2, space="PSUM") as tps:

                for t in range(L):
                    cur, nxt = t % 2, (t + 1) % 2
                    xps = xp_t[t // 8]
                    blk = 32 * ((t % 8) // 2)
                    o = 16 * ((t % 8) % 2)

                    P = gps.tile([128, 512], FP, tag="P", name="P")
                    for j in range(4):
                        # xpart inject: b=0 strip [32j, 32j+16)
                        nc.tensor.matmul(
                            P[32 * j:32 * j + 16, 0:HH],
                            idsel[blk:blk + 32, o:o + 16],
                            xps[blk:blk + 32, 512 * j:512 * j + HH],
                            start=True, stop=False,
                            tile_position=(blk, 32 * j))
                        # b=1 strip via 32-wide zero-padded stationary
                        nc.tensor.matmul(
                            P[32 * j:32 * j + 32, 0:HH],
                            idsel[blk:blk + 32, 32 + 2 * o:32 + 2 * o + 32],
                            xps[blk:blk + 32, 512 * j + HH:512 * (j + 1)],
                            start=False, stop=False,
                            tile_position=(blk, 32 * j))
                        for k in range(4):
                            nc.tensor.matmul(
                                P[32 * j:32 * j + 16, 0:HH],
                                hT_t[cur][:, k, 16:32],
                                W_t[4 + k][:, 512 * j:512 * j + HH],
                                start=False, stop=(k == 3),
                                tile_position=(0, 32 * j))
                            nc.tensor.matmul(
                                P[32 * j:32 * j + 32, 0:HH],
                                hT_t[cur][:, k, 0:32],
                                W_t[4 + k][:, 512 * j + HH:512 * (j + 1)],
                                start=False, stop=(k == 3),
                                tile_position=(0, 32 * j))

                    # activations (sigma-form): f,i,o sigmoid; g tanh.
                    # DVE tensor_tensor needs equal input base partitions, so
                    # g lands at base 32 (pairs with i) and tanh(c) at base
                    # 64 (pairs with o).
                    T = ch.tile([96, HH], FP, tag="T", name="T")
                    nc.scalar.activation(T[0:96, :], P[0:96, 0:HH], AF.Sigmoid)
                    Tg = ch.tile([64, HH], FP, tag="Tg", name="Tg")
                    nc.scalar.activation(Tg[32:64, :], P[96:128, 0:HH], AF.Tanh)

                    m1 = ch.tile([2 * B, HH], FP, tag="m1", name="m1")
                    nc.vector.tensor_mul(m1[:, :], T[0:32, :], c_t[cur][:, :])
                    m2 = ch.tile([2 * B, HH], FP, tag="m2", name="m2")
                    nc.vector.tensor_mul(m2[:, :], T[32:64, :], Tg[32:64, :])
                    nc.vector.tensor_add(c_t[nxt][:, :], m1[:, :], m2[:, :])
                    TH = ch.tile([96, HH], FP, tag="TH", name="TH")
                    nc.scalar.activation(TH[64:96, :], c_t[nxt][:, :], AF.Tanh)
                    Hh = hb.tile([2 * B, HH], FP, tag="Hh", name="Hh")
                    nc.vector.tensor_mul(Hh[:, :], T[64:96, :], TH[64:96, :])

                    nc.sync.dma_start(out_h[t, :, :], Hh[:, :])

                    # transpose [32,128] blocks: block c yields hT chunks c
                    # (cols 0:16, from h half 0) and c+2 (cols 16:32, half 1)
                    pst = tps.tile([128, 2, 256], FP, tag="pst", name="pst")
                    for c in range(2):
                        nc.tensor.transpose(
                            pst[:, c, 0:32],
                            Hh[:, 128 * c:128 * (c + 1)],
                            ident32[:, :])
                    nc.vector.tensor_copy(
                        hT_t[nxt][:, 0:2, 16:32], pst[:, 0:2, 0:16])
                    nc.vector.tensor_copy(
                        hT_t[nxt][:, 2:4, 16:32], pst[:, 0:2, 16:32])
    nc.compile()
    return nc


def _make_consts():
    blk = np.zeros((32, 96), np.float32)
    I16 = np.eye(16, dtype=np.float32)
    blk[0:16, 0:16] = I16          # S0
    blk[16:32, 16:32] = I16        # S16
    blk[0:16, 48:64] = I16         # [0|S0] cols 32:64
    blk[16:32, 80:96] = I16        # [0|S16] cols 64:96
    idsel = np.tile(blk, (4, 1))   # replicated at partition bases 0/32/64/96
    return idsel.astype(ml_dtypes.bfloat16), np.eye(32, dtype=np.float32)


def _host_prepare(x_full, weights, direction, bslice, consts):
    xs = x_full[bslice]
    if direction == "bw":
        xs = xs[:, ::-1, :]
    xT = np.ascontiguousarray(xs.transpose(2, 1, 0).reshape(D, TOK))
    Wc = np.concatenate(
        [weights[f"W_{direction}_{n}"].T for n in "fiog"], axis=1)
    bc = np.concatenate(
        [weights[f"b_{direction}_{n}"] for n in "fiog"])[None, :]
    bf = ml_dtypes.bfloat16
    return {"xT": xT.astype(bf),
            "W": np.ascontiguousarray(Wc).astype(bf),
            "bias": np.ascontiguousarray(bc).astype(bf),
            "idsel": consts[0], "id32": consts[1]}


def kernel(**inputs):
    global _CACHED_NC
    inputs = {k: np.asarray(v) for k, v in inputs.items()}
    x = inputs["x"]
    Bx, Lx, _ = x.shape
    assert (Bx, Lx) == (64, L)

    if _CACHED_NC is None:
        _CACHED_NC = _build()
    nc = _CACHED_NC

    consts = _make_consts()
    in_maps = []
    meta = []
    for ci in range(8):
        d = "fw" if ci < 4 else "bw"
        bs = (ci % 4) * B
        in_maps.append(_host_prepare(x, inputs, d, slice(bs, bs + B), consts))
        meta.append((d, bs))

    res = run_bass_kernel_spmd(nc, in_maps, core_ids=list(range(8)))

    hf = np.zeros((L, Bx, H), np.float32)
    hb = np.zeros((L, Bx, H), np.float32)
    for ci in range(8):
        d, bs = meta[ci]
        oh = res.results[ci]["out_h"]  # (L, 32, 256): [16b+beta, n] -> H=256b+n
        oh = np.asarray(oh, np.float32).reshape(L, 2, B, HH)
        full = oh.transpose(0, 2, 1, 3).reshape(L, B, H)  # (L, 16, 512)
        if d == "fw":
            hf[:, bs:bs + B, :] = full
        else:
            hb[:, bs:bs + B, :] = full[::-1]

    flat = np.concatenate([hf.reshape(-1, H), hb.reshape(-1, H)], axis=1)
    return flat.reshape(Bx, Lx, 2 * H).astype(np.float32)


# revision 31
# speedup vs baseline: 1.9120x; 1.4151x over previous
"""BiLSTM (B=64, L=256, D=512, H=512) on 8 Trainium2 NeuronCores.

8 cores = 2 directions x 4 batch-slices of 16; weights replicated per
direction; backward cores get time-reversed x so one SPMD program serves
all cores.

vs baseline:
  - all matmul operands bf16 (fp32 matmuls are 2-pass on TRN2; bf16 is
    single-pass); fp32 PSUM accumulation; cell state c kept fp32.
  - xpart (x @ Wx.T + b for all tokens) stays resident in SBUF (16 MB
    bf16) instead of a 64 MB DRAM round-trip; the per-step row selection
    uses K=32 shifted-identity stationaries so the moving operand is
    always 32-partition aligned.
  - phase-1 is interleaved into the recurrence: one gate-quarter GEMM
    quantum per even step runs in the PE idle window of the step's
    activation/vector chain (12-quantum prologue keeps a 3-tile lead).
  - next step's xpart inject is hoisted right after this step's Wh
    matmuls, so it also runs during the chain.
  - Wh matmuls are issued strips-innermost so the 4 col groups stream
    concurrently.
  - PSUM writes keep the baseline's race-free pattern: each 32-row col
    group is written by exactly one 16-row strip (start=True inject then
    accumulates); no overlapping-row matmuls.
"""

import numpy as np
import ml_dtypes

from concourse import tile, mybir, bacc
from concourse.bass_utils import run_bass_kernel_spmd

FP = mybir.dt.float32
BF = mybir.dt.bfloat16
AF = mybir.ActivationFunctionType

B = 16        # local batch per core
L = 256       # timesteps
D = 512       # input dim
H = 512       # hidden
NG = 4 * H    # gate width
TOK = L * B   # tokens per core
NM = TOK // 128

_CACHED_NC = None


def _build():
    nc = bacc.Bacc("TRN2", target_bir_lowering=False, debug=False)

    xT = nc.dram_tensor("xT", [D, TOK], BF, kind="ExternalInput").ap()
    W = nc.dram_tensor("W", [D + H, NG], BF, kind="ExternalInput").ap()
    bias = nc.dram_tensor("bias", [1, NG], BF, kind="ExternalInput").ap()
    # row-selector stationaries, replicated at partition bases 0/32/64/96:
    # cols 0:16 = S0, cols 16:32 = S16 (S_o picks moving rows o..o+16)
    idsel_d = nc.dram_tensor("idsel", [128, 32], BF, kind="ExternalInput").ap()
    id16_d = nc.dram_tensor("id16", [16, 16], FP, kind="ExternalInput").ap()
    out_h = nc.dram_tensor("out_h", [L, B, H], FP, kind="ExternalOutput").ap()

    with tile.TileContext(nc, trace_sim=False) as tc:
        with tc.tile_pool(name="wpool", bufs=1) as wpool, \
             tc.tile_pool(name="xppool", bufs=1) as xppool, \
             tc.tile_pool(name="cpool", bufs=1) as cpool:
            W_t = []
            for k in range(8):
                wt = wpool.tile([128, NG], BF, tag=f"w{k}", name=f"w{k}")
                nc.sync.dma_start(wt[:], W[128 * k:128 * (k + 1), :])
                W_t.append(wt)
            bias_t = wpool.tile([1, NG], BF)
            nc.sync.dma_start(bias_t[:], bias[:, :])
            ones_t = cpool.tile([1, 128], BF)
            nc.vector.memset(ones_t[:, :], 1.0)
            idsel = cpool.tile([128, 32], BF, tag="idsel", name="idsel")
            nc.sync.dma_start(idsel[:, :], idsel_d[:, :])
            ident16 = cpool.tile([16, 16], FP, tag="ident16", name="ident16")
            nc.sync.dma_start(ident16[:, :], id16_d[:, :])

            # persistent SBUF xpart tiles (32 x [128, 2048] bf16)
            xp_t = [xppool.tile([128, NG], BF, tag=f"xp{m}", name=f"xp{m}")
                    for m in range(NM)]

            # state tiles (ping-pong); hT [128, 4, 16] chunk-major
            c_t = [cpool.tile([B, H], FP, tag=f"c{i}", name=f"c{i}")
                   for i in range(2)]
            hT_t = [cpool.tile([128, 4, 16], BF, tag=f"hT{i}", name=f"hT{i}")
                    for i in range(2)]
            nc.vector.memset(c_t[0][:, :], 0.0)
            nc.vector.memset(hT_t[0][:, :, :], 0.0)
            nc.vector.memset(hT_t[1][:, :, :], 0.0)

            with tc.tile_pool(name="p1x", bufs=3) as p1x, \
                 tc.tile_pool(name="p1ps", bufs=2, space="PSUM") as p1ps, \
                 tc.tile_pool(name="ch", bufs=3) as ch, \
                 tc.tile_pool(name="hb", bufs=2) as hb, \
                 tc.tile_pool(name="gps", bufs=2, space="PSUM") as gps, \
                 tc.tile_pool(name="tps", bufs=2, space="PSUM") as tps:

                xm_tiles = {}

                def emit_p1(q):
                    m, n = q // 4, q % 4
                    if n == 0:
                        xm = p1x.tile([128, 4, 128], BF, tag="xm", name="xm")
                        xm_tiles[m] = xm
                        for k in range(4):
                            nc.sync.dma_start(
                                xm[:, k, :],
                                xT[128 * k:128 * (k + 1),
                                   128 * m:128 * (m + 1)])
                    xm = xm_tiles[m]
                    ps = p1ps.tile([128, 512], FP, tag="ps1", name="ps1")
                    for k in range(4):
                        nc.tensor.matmul(
                            ps[:, :], xm[:, k, :],
                            W_t[k][:, 512 * n:512 * (n + 1)],
                            start=(k == 0), stop=False)
                    nc.tensor.matmul(
                        ps[:, :], ones_t[:, :],
                        bias_t[:, 512 * n:512 * (n + 1)],
                        start=False, stop=True)
                    nc.vector.tensor_copy(
                        xp_t[m][:, 512 * n:512 * (n + 1)], ps[:, :])

                def emit_inject(t):
                    # gate strip j at partitions 32j..32j+16; K=32 selector
                    # keeps the moving xpart rows 32-partition aligned
                    xps = xp_t[t // 8]
                    blk = 32 * ((t % 8) // 2)
                    o = 16 * ((t % 8) % 2)
                    P = gps.tile([128, 512], FP, tag="P", name="P")
                    for j in range(4):
                        nc.tensor.matmul(
                            P[32 * j:32 * j + 16, :],
                            idsel[blk:blk + 32, o:o + 16],
                            xps[blk:blk + 32, 512 * j:512 * (j + 1)],
                            start=True, stop=False,
                            tile_position=(blk, 32 * j))
                    return P

                for q in range(12):
                    emit_p1(q)
                P_cur = emit_inject(0)

                for t in range(L):
                    cur, nxt = t % 2, (t + 1) % 2
                    P = P_cur

                    # Wh matmuls, strips innermost -> concurrent col groups
                    for k in range(4):
                        for j in range(4):
                            nc.tensor.matmul(
                                P[32 * j:32 * j + 16, :],
                                hT_t[cur][:, k, :],
                                W_t[4 + k][:, 512 * j:512 * (j + 1)],
                                start=False, stop=(k == 3),
                                tile_position=(0, 32 * j))

                    # next step's inject runs on PE during this step's chain
                    if t + 1 < L:
                        P_cur = emit_inject(t + 1)

                    # strips: f@0:16, i@32:48, o@64:80, g@96:112
                    T = ch.tile([80, H], FP, tag="T", name="T")
                    nc.scalar.activation(T[0:80, :], P[0:80, :], AF.Sigmoid)
                    Tg = ch.tile([48, H], FP, tag="Tg", name="Tg")
                    nc.scalar.activation(Tg[32:48, :], P[96:112, :], AF.Tanh)

                    m1 = ch.tile([B, H], FP, tag="m1", name="m1")
                    nc.vector.tensor_mul(m1[:, :], T[0:16, :], c_t[cur][:, :])
                    m2 = ch.tile([B, H], FP, tag="m2", name="m2")
                    nc.vector.tensor_mul(m2[:, :], T[32:48, :], Tg[32:48, :])
                    nc.vector.tensor_add(c_t[nxt][:, :], m1[:, :], m2[:, :])
                    TH = ch.tile([80, H], FP, tag="TH", name="TH")
                    nc.scalar.activation(TH[64:80, :], c_t[nxt][:, :], AF.Tanh)
                    Hh = hb.tile([B, H], FP, tag="Hh", name="Hh")
                    nc.vector.tensor_mul(Hh[:, :], T[64:80, :], TH[64:80, :])

                    nc.sync.dma_start(out_h[t, :, :], Hh[:, :])

                    # transpose h chunks [16,128] -> [128,16]; copies feed
                    # next step's stationaries chunk-by-chunk
                    pst = tps.tile([128, 4, 128], FP, tag="pst", name="pst")
                    for kc in range(4):
                        nc.tensor.transpose(
                            pst[:, kc, 0:16],
                            Hh[:, 128 * kc:128 * (kc + 1)],
                            ident16[:, :])
                        nc.vector.tensor_copy(
                            hT_t[nxt][:, kc, :], pst[:, kc, 0:16])

                    # one phase-1 quantum per even step fills leftover PE
                    # idle in the chain window
                    if t % 2 == 0 and 12 + t // 2 < 4 * NM:
                        emit_p1(12 + t // 2)
    nc.compile()
    return nc


def _make_consts():
    blk = np.zeros((32, 32), np.float32)
    I16 = np.eye(16, dtype=np.float32)
    blk[0:16, 0:16] = I16          # S0
    blk[16:32, 16:32] = I16        # S16
    idsel = np.tile(blk, (4, 1))   # replicated at partition bases 0/32/64/96
    return idsel.astype(ml_dtypes.bfloat16), I16.copy()


def _host_prepare(x_full, weights, direction, bslice, consts):
    xs = x_full[bslice]
    if direction == "bw":
        xs = xs[:, ::-1, :]
    xT = np.ascontiguousarray(xs.transpose(2, 1, 0).reshape(D, TOK))
    Wc = np.concatenate(
        [weights[f"W_{direction}_{n}"].T for n in "fiog"], axis=1)
    bc = np.concatenate(
        [weights[f"b_{direction}_{n}"] for n in "fiog"])[None, :]
    bf = ml_dtypes.bfloat16
    return {"xT": xT.astype(bf),
            "W": np.ascontiguousarray(Wc).astype(bf),
            "bias": np.ascontiguousarray(bc).astype(bf),
            "idsel": consts[0], "id16": consts[1]}


def kernel(**inputs):
    global _CACHED_NC
    inputs = {k: np.asarray(v) for k, v in inputs.items()}
    x = inputs["x"]
    Bx, Lx, _ = x.shape
    assert (Bx, Lx) == (64, L)

    if _CACHED_NC is None:
        _CACHED_NC = _build()
    nc = _CACHED_NC

    consts = _make_consts()
    in_maps = []
    meta = []
    for ci in range(8):
        d = "fw" if ci < 4 else "bw"
        bs = (ci % 4) * B
        in_maps.append(_host_prepare(x, inputs, d, slice(bs, bs + B), consts))
        meta.append((d, bs))

    res = run_bass_kernel_spmd(nc, in_maps, core_ids=list(range(8)))

    hf = np.zeros((L, Bx, H), np.float32)
    hb = np.zeros((L, Bx, H), np.float32)
    for ci in range(8):
        d, bs = meta[ci]
        oh = np.asarray(res.results[ci]["out_h"], np.float32)  # (L, 16, 512)
        if d == "fw":
            hf[:, bs:bs + B, :] = oh
        else:
            hb[:, bs:bs + B, :] = oh[::-1]

    flat = np.concatenate([hf.reshape(-1, H), hb.reshape(-1, H)], axis=1)
    return flat.reshape(Bx, Lx, 2 * H).astype(np.float32)
